# revision 10
# baseline (speedup 1.0000x reference)
"""MoE routing kernel for Trainium2, sharded across 8 NeuronCores.

Math (reference): top-2 gated MoE with 6 experts,
  out[b,o] = sum_e gates[b,e] * (relu(x @ w1[e] + b1[e]) @ w2[e] + b2[e])[b,o]

Sharding: the expert computation decomposes into E*H = 384 independent
"hidden units" (unit (e,j) touches only w1[e,:,j] and w2[e,j,:]).  Each of
the 8 cores gets 48 units and produces a gate-weighted partial output
[32,1000]; the partials are summed on the host (the unshard step for a
partial-sum sharding).  Gating (logits -> top2 -> softmax -> dense gates)
is replicated on every core; a per-core 0/1 selection matrix maps gate
columns onto that core's units, so one uniform SPMD program serves all
cores.

Perf notes:
- x^T, gate_w, gate_b, b1 and the selection matrix ride in ONE packed
  [128,361] DMA so the gating chain starts as early as possible; w1 and
  the repacked w2 overlap on the two HW DGE queues (SP + Activation).
- top-2 via the DVE `max` sort unit (one instruction) instead of a
  max/mask/max chain; softmax normalization is deferred into the
  PSUM->SBUF output copies (per-partition 1/den scale), which also
  normalizes the b2 gate rows for free.
- w2 is repacked to [128,500] (column halves stacked at a 64-partition
  offset) so its DMA is dense and both matmul slices are 32-aligned.
- the [32,1000] output is produced in 4 matmul chunks whose copies and
  DMAs pipeline behind the PE, alternating the two DMA queues.
"""

import numpy as np

B, D, H, O, E = 32, 1024, 64, 1000, 6
NCORE = 8
P = 128
KC = D // P          # 8 k-chunks for the D contraction
U = (E * H) // NCORE  # 48 hidden units per core
M = U + 2             # +2 rows carry the b2 bias contributions

XGW = 311 + M        # packed input width: xt | gws | gbr | b1 | sel
GOFF = KC * B        # 256: gate_w chunks
BOFF = 304           # gate_b row
B1OFF = 310          # b1 column
SOFF = 311           # sel block

_CACHE = {}


def _build_nc():
    import concourse.bacc as bacc
    import concourse.mybir as mybir
    import concourse.tile as tile
    from concourse.masks import make_identity

    f32 = mybir.dt.float32
    Alu = mybir.AluOpType
    Act = mybir.ActivationFunctionType

    nc = bacc.Bacc(
        "TRN2",
        target_bir_lowering=False,
        debug=False,
        enable_asserts=False,
        num_devices=NCORE,
    )

    def din(name, shape):
        return nc.dram_tensor(name, shape, f32, kind="ExternalInput").ap()

    t_xg = din("xg", [P, XGW])        # xt | gws | gbr | b1 | sel (see _prep)
    t_w1s = din("w1s", [P, KC * U])   # per-core w1 cols: [p, k*U+j] = w1[e_j, k*P+p, h_j]
    t_w2p = din("w2p", [P, O // 2])   # rows 0:M = w2s[:, :500]; rows 64:64+M = w2s[:, 500:]
    t_out = nc.dram_tensor("out", [B, O], f32, kind="ExternalOutput").ap()

    with tile.TileContext(nc) as tc:
        with (
            tc.tile_pool(name="sb", bufs=1) as sb,
            tc.tile_pool(name="ps", bufs=1, space="PSUM") as ps,
        ):
            # --- loads: xg first (feeds the whole gating chain), w1s second
            # on the SP queue; w2p alone on the Activation queue ---
            xg = sb.tile([P, XGW], f32)
            nc.sync.dma_start(xg[:], t_xg[:])
            w1s = sb.tile([P, KC * U], f32)
            nc.sync.dma_start(w1s[:], t_w1s[:])
            w2p = sb.tile([P, O // 2], f32)
            nc.sync.dma_start(w2p[:], t_w2p[:])

            gbr = xg[0:1, BOFF:BOFF + E]
            b1s = xg[0:U, B1OFF:B1OFF + 1]
            sel = xg[0:E, SOFF:SOFF + M]

            # small constants, prepared on idle engines during the DMAs
            ones = sb.tile([1, B], f32)
            nc.gpsimd.memset(ones[:], 1.0)
            lgs = sb.tile([B, 8], f32)   # logits padded to 8 for the sort unit
            nc.gpsimd.memset(lgs[:], -1e30)
            R = sb.tile([M, B], f32)     # relu(h+b1) rows; 1.0 in the b2 slots
            nc.gpsimd.memset(R[:], 1.0)
            ident = sb.tile([B, B], f32)
            make_identity(nc, ident[:])

            # logits [B, E] = x @ gate_w + gate_b  (bias via a K=1 ones matmul)
            lg = ps.tile([B, E], f32)
            for k in range(KC):
                nc.tensor.matmul(
                    lg[:],
                    xg[:, k * B:(k + 1) * B],
                    xg[:, GOFF + k * E:GOFF + (k + 1) * E],
                    start=(k == 0),
                    stop=False,
                )
            nc.tensor.matmul(lg[:], ones[:], gbr, start=False, stop=True)

            # hT [U, B] = (w1 cols)^T @ x^T  (PE runs this while the gating
            # chain occupies DVE/ACT)
            ht = ps.tile([U, B], f32)
            for k in range(KC):
                nc.tensor.matmul(
                    ht[:],
                    w1s[:, k * U:(k + 1) * U],
                    xg[:, k * B:(k + 1) * B],
                    start=(k == 0),
                    stop=(k == KC - 1),
                )

            # top-2 gating.  mx[:,0:2] = two largest logits per row (DVE sort
            # unit); wgt = exp(logits) masked to the top-2 (unnormalized --
            # the 1/den softmax scale is applied to the final output instead,
            # which also normalizes the b2 gate rows).
            nc.vector.tensor_copy(lgs[:, 0:E], lg[:])
            ex = sb.tile([B, E], f32)
            nc.scalar.activation(ex[:], lg[:], Act.Exp, scale=1.0)
            mx = sb.tile([B, 8], f32)
            nc.vector.max(mx[:], lgs[:])
            ind = sb.tile([B, E], f32)   # 1.0 at top-2 entries
            nc.vector.tensor_scalar(ind[:], lgs[:, 0:E], mx[:, 1:2], None, Alu.is_ge)
            wgt = sb.tile([B, E], f32)
            nc.vector.tensor_mul(wgt[:], ind[:], ex[:])
            den = sb.tile([B, 1], f32)
            nc.vector.tensor_reduce(den[:], wgt[:], axis=mybir.AxisListType.X, op=Alu.add)
            rden = sb.tile([B, 1], f32)
            nc.vector.reciprocal(rden[:], den[:])

            # GT [M, B] = sel^T @ wgt^T : per-unit (unnormalized) gate rows
            gtp = ps.tile([E, B], f32)
            nc.tensor.transpose(gtp[:], wgt[:], ident[:])
            gt = sb.tile([E, B], f32)
            nc.vector.tensor_copy(gt[:], gtp[:])
            GT = ps.tile([M, B], f32)
            nc.tensor.matmul(GT[:], sel, gt[:], start=True, stop=True)

            # R rows :U = relu(hT + b1) (b2 slots stay at their memset 1.0);
            # MT = GT . R is the gated hidden matrix, written at base
            # partitions 0 AND 64 so each output matmul's lhsT shares a base
            # partition with its w2p slice
            nc.vector.tensor_scalar(R[:U, :], ht[:], b1s, 0.0, Alu.add, Alu.max)
            MT = sb.tile([P, B], f32)
            nc.vector.tensor_mul(MT[0:M, :], GT[:], R[:])
            nc.vector.tensor_mul(MT[64:64 + M, :], GT[:], R[:])

            # partial out [B, O] = (MT^T @ w2) * rden, in 4 pipelined chunks
            osb = sb.tile([B, O], f32)
            NC4 = O // 4
            for i in range(4):
                r0 = 0 if i < 2 else 64
                c0 = (i % 2) * NC4
                op = ps.tile([B, NC4], f32, tag=f"op{i}", name=f"op{i}")
                nc.tensor.matmul(
                    op[:],
                    MT[r0:r0 + M, :],
                    w2p[r0:r0 + M, c0:c0 + NC4],
                    start=True,
                    stop=True,
                )
                o0 = i * NC4
                nc.vector.tensor_scalar_mul(osb[:, o0:o0 + NC4], op[:], rden[:])
                eng = nc.sync if i % 2 == 0 else nc.scalar
                eng.dma_start(t_out[:, o0:o0 + NC4], osb[:, o0:o0 + NC4])

    nc.compile()
    return nc


def _get_nc():
    if "nc" not in _CACHE:
        _CACHE["nc"] = _build_nc()
    return _CACHE["nc"]


def _prep_core_inputs(c, x, gate_w, gate_b, w1, b1, w2, b2):
    us = np.arange(U * c, U * (c + 1))
    es = us // H
    hs = us % H

    xg = np.zeros((P, XGW), np.float32)
    xg[:, 0:KC * B] = x.reshape(B, KC, P).transpose(2, 1, 0).reshape(P, KC * B)
    xg[:, GOFF:GOFF + KC * E] = (
        gate_w.reshape(KC, P, E).transpose(1, 0, 2).reshape(P, KC * E)
    )
    xg[0, BOFF:BOFF + E] = gate_b
    xg[0:U, B1OFF] = b1[es, hs]

    sel = np.zeros((E, M), np.float32)
    sel[es, np.arange(U)] = 1.0

    w2s = np.zeros((M, O), np.float32)
    w2s[:U] = w2[es, hs, :]
    owned = [e for e in range(E) if U * c <= e * H < U * (c + 1)]
    for slot, e in enumerate(owned):
        w2s[U + slot] = b2[e]
        sel[e, U + slot] = 1.0
    xg[0:E, SOFF:SOFF + M] = sel

    w2p = np.zeros((P, O // 2), np.float32)
    w2p[0:M] = w2s[:, :O // 2]
    w2p[64:64 + M] = w2s[:, O // 2:]

    w1c = w1[es, :, hs]  # [U, D]
    w1s = np.ascontiguousarray(
        w1c.T.reshape(KC, P, U).transpose(1, 0, 2).reshape(P, KC * U)
    )

    return {"xg": xg, "w1s": w1s, "w2p": w2p}


def _run(inputs, trace=False, **kw):
    from concourse.bass_utils import run_bass_kernel_spmd

    args = {
        k: np.ascontiguousarray(np.asarray(v, dtype=np.float32))
        for k, v in inputs.items()
    }
    nc = _get_nc()
    in_maps = [_prep_core_inputs(c, **args) for c in range(NCORE)]
    res = run_bass_kernel_spmd(
        nc, in_maps, core_ids=list(range(NCORE)), trace=trace, **kw
    )
    out = np.zeros((B, O), np.float32)
    for r in res.results:
        out += r["out"]
    return out, res


def kernel(**inputs):
    out, _ = _run(inputs)
    return out


# revision 11
# speedup vs baseline: 1.0335x; 1.0335x over previous
"""MoE routing kernel for Trainium2, sharded across 8 NeuronCores.

Math (reference): top-2 gated MoE with 6 experts,
  out[b,o] = sum_e gates[b,e] * (relu(x @ w1[e] + b1[e]) @ w2[e] + b2[e])[b,o]

Sharding: the expert computation decomposes into E*H = 384 independent
"hidden units" (unit (e,j) touches only w1[e,:,j] and w2[e,j,:]).  Each of
the 8 cores gets 48 units and produces a gate-weighted partial output
[32,1000]; the partials are summed on the host (the unshard step for a
partial-sum sharding).  Gating (logits -> top2 -> softmax -> dense gates)
is replicated on every core; a per-core 0/1 selection matrix maps gate
columns onto that core's units, so one uniform SPMD program serves all
cores.

Perf notes:
- x^T, gate_w, gate_b, b1 and the selection matrix ride in ONE packed
  [128,361] DMA so the gating chain starts as early as possible; w1 and
  the repacked w2 overlap on the two HW DGE queues (SP + Activation).
- top-2 via the DVE `max` sort unit (one instruction) instead of a
  max/mask/max chain; softmax normalization is deferred into the
  PSUM->SBUF output copies (per-partition 1/den scale), which also
  normalizes the b2 gate rows for free.
- w2 is repacked to [128,500] (column halves stacked at a 64-partition
  offset) so its DMA is dense and both matmul slices are 32-aligned.
- the [32,1000] output is produced in 4 matmul chunks whose copies and
  DMAs pipeline behind the PE, alternating the two DMA queues.
"""

import numpy as np

B, D, H, O, E = 32, 1024, 64, 1000, 6
NCORE = 8
P = 128
KC = D // P          # 8 k-chunks for the D contraction
U = (E * H) // NCORE  # 48 hidden units per core
M = U + 2             # +2 rows carry the b2 bias contributions

XGW = 311 + M        # packed input width: xt | gws | gbr | b1 | sel
GOFF = KC * B        # 256: gate_w chunks
BOFF = 304           # gate_b row
B1OFF = 310          # b1 column
SOFF = 311           # sel block

_CACHE = {}


def _build_nc():
    import concourse.bacc as bacc
    import concourse.mybir as mybir
    import concourse.tile as tile
    from concourse.masks import make_identity

    f32 = mybir.dt.float32
    Alu = mybir.AluOpType
    Act = mybir.ActivationFunctionType

    nc = bacc.Bacc(
        "TRN2",
        target_bir_lowering=False,
        debug=False,
        enable_asserts=False,
        num_devices=NCORE,
    )

    def din(name, shape):
        return nc.dram_tensor(name, shape, f32, kind="ExternalInput").ap()

    t_xg = din("xg", [P, XGW])        # xt | gws | gbr | b1 | sel (see _prep)
    t_w1s = din("w1s", [P, KC * U])   # per-core w1 cols: [p, k*U+j] = w1[e_j, k*P+p, h_j]
    t_w2p = din("w2p", [P, O // 2])   # rows 0:M = w2s[:, :500]; rows 64:64+M = w2s[:, 500:]
    t_out = nc.dram_tensor("out", [B, O], f32, kind="ExternalOutput").ap()

    with tile.TileContext(nc) as tc:
        with (
            tc.tile_pool(name="sb", bufs=1) as sb,
            tc.tile_pool(name="ps", bufs=1, space="PSUM") as ps,
        ):
            # --- loads: xg first (feeds the whole gating chain), w1s second
            # on the SP queue; w2p alone on the Activation queue ---
            xg = sb.tile([P, XGW], f32)
            nc.sync.dma_start(xg[:], t_xg[:])
            w1s = sb.tile([P, KC * U], f32)
            nc.sync.dma_start(w1s[:], t_w1s[:])
            w2p = sb.tile([P, O // 2], f32)
            nc.sync.dma_start(w2p[:], t_w2p[:])

            gbr = xg[0:1, BOFF:BOFF + E]
            b1s = xg[0:U, B1OFF:B1OFF + 1]
            sel = xg[0:E, SOFF:SOFF + M]

            # small constants, prepared on idle engines during the DMAs
            ones = sb.tile([1, B], f32)
            nc.gpsimd.memset(ones[:], 1.0)
            lgs = sb.tile([B, 8], f32)   # logits padded to 8 for the sort unit
            nc.gpsimd.memset(lgs[:], -1e30)
            R = sb.tile([M, B], f32)     # relu(h+b1) rows; 1.0 in the b2 slots
            nc.gpsimd.memset(R[:], 1.0)
            ident = sb.tile([B, B], f32)
            make_identity(nc, ident[:])

            # logits [B, E] = x @ gate_w + gate_b  (bias via a K=1 ones matmul)
            lg = ps.tile([B, E], f32)
            for k in range(KC):
                nc.tensor.matmul(
                    lg[:],
                    xg[:, k * B:(k + 1) * B],
                    xg[:, GOFF + k * E:GOFF + (k + 1) * E],
                    start=(k == 0),
                    stop=False,
                )
            nc.tensor.matmul(lg[:], ones[:], gbr, start=False, stop=True)

            # hT [U, B] = (w1 cols)^T @ x^T  (PE runs this while the gating
            # chain occupies DVE/ACT)
            ht = ps.tile([U, B], f32)
            for k in range(KC):
                nc.tensor.matmul(
                    ht[:],
                    w1s[:, k * U:(k + 1) * U],
                    xg[:, k * B:(k + 1) * B],
                    start=(k == 0),
                    stop=(k == KC - 1),
                )

            # top-2 gating.  mx[:,0:2] = two largest logits per row (DVE sort
            # unit); wgt = exp(logits) masked to the top-2 (unnormalized --
            # the 1/den softmax scale is applied to the final output instead,
            # which also normalizes the b2 gate rows).
            nc.vector.tensor_copy(lgs[:, 0:E], lg[:])
            ex = sb.tile([B, E], f32)
            nc.scalar.activation(ex[:], lg[:], Act.Exp, scale=1.0)
            mx = sb.tile([B, 8], f32)
            nc.vector.max(mx[:], lgs[:])
            ind = sb.tile([B, E], f32)   # 1.0 at top-2 entries
            nc.vector.tensor_scalar(ind[:], lgs[:, 0:E], mx[:, 1:2], None, Alu.is_ge)
            wgt = sb.tile([B, E], f32)
            nc.vector.tensor_mul(wgt[:], ind[:], ex[:])
            den = sb.tile([B, 1], f32)
            nc.vector.tensor_reduce(den[:], wgt[:], axis=mybir.AxisListType.X, op=Alu.add)
            rden = sb.tile([B, 1], f32)
            nc.vector.reciprocal(rden[:], den[:])

            # GT [M, B] = sel^T @ wgt^T : per-unit (unnormalized) gate rows
            gtp = ps.tile([E, B], f32)
            nc.tensor.transpose(gtp[:], wgt[:], ident[:])
            gt = sb.tile([E, B], f32)
            nc.vector.tensor_copy(gt[:], gtp[:])
            GT = ps.tile([M, B], f32)
            nc.tensor.matmul(GT[:], sel, gt[:], start=True, stop=True)

            # R rows :U = relu(hT + b1) (b2 slots stay at their memset 1.0);
            # MT = GT . R is the gated hidden matrix, written at base
            # partitions 0 AND 64 so each output matmul's lhsT shares a base
            # partition with its w2p slice
            nc.gpsimd.tensor_scalar(R[:U, :], ht[:], b1s, 0.0, Alu.add, Alu.max)
            MT = sb.tile([P, B], f32)
            nc.vector.tensor_mul(MT[0:M, :], GT[:], R[:])
            nc.vector.tensor_mul(MT[64:64 + M, :], GT[:], R[:])

            # partial out [B, O] = (MT^T @ w2) * rden, in 4 pipelined chunks
            osb = sb.tile([B, O], f32)
            NC4 = O // 4
            for i in range(4):
                r0 = 0 if i < 2 else 64
                c0 = (i % 2) * NC4
                op = ps.tile([B, NC4], f32, tag=f"op{i}", name=f"op{i}")
                nc.tensor.matmul(
                    op[:],
                    MT[r0:r0 + M, :],
                    w2p[r0:r0 + M, c0:c0 + NC4],
                    start=True,
                    stop=True,
                )
                o0 = i * NC4
                nc.vector.tensor_scalar_mul(osb[:, o0:o0 + NC4], op[:], rden[:])
                eng = nc.sync if i % 2 == 0 else nc.scalar
                eng.dma_start(t_out[:, o0:o0 + NC4], osb[:, o0:o0 + NC4])

    nc.compile()
    return nc


def _get_nc():
    if "nc" not in _CACHE:
        _CACHE["nc"] = _build_nc()
    return _CACHE["nc"]


def _prep_core_inputs(c, x, gate_w, gate_b, w1, b1, w2, b2):
    us = np.arange(U * c, U * (c + 1))
    es = us // H
    hs = us % H

    xg = np.zeros((P, XGW), np.float32)
    xg[:, 0:KC * B] = x.reshape(B, KC, P).transpose(2, 1, 0).reshape(P, KC * B)
    xg[:, GOFF:GOFF + KC * E] = (
        gate_w.reshape(KC, P, E).transpose(1, 0, 2).reshape(P, KC * E)
    )
    xg[0, BOFF:BOFF + E] = gate_b
    xg[0:U, B1OFF] = b1[es, hs]

    sel = np.zeros((E, M), np.float32)
    sel[es, np.arange(U)] = 1.0

    w2s = np.zeros((M, O), np.float32)
    w2s[:U] = w2[es, hs, :]
    owned = [e for e in range(E) if U * c <= e * H < U * (c + 1)]
    for slot, e in enumerate(owned):
        w2s[U + slot] = b2[e]
        sel[e, U + slot] = 1.0
    xg[0:E, SOFF:SOFF + M] = sel

    w2p = np.zeros((P, O // 2), np.float32)
    w2p[0:M] = w2s[:, :O // 2]
    w2p[64:64 + M] = w2s[:, O // 2:]

    w1c = w1[es, :, hs]  # [U, D]
    w1s = np.ascontiguousarray(
        w1c.T.reshape(KC, P, U).transpose(1, 0, 2).reshape(P, KC * U)
    )

    return {"xg": xg, "w1s": w1s, "w2p": w2p}


def _run(inputs, trace=False, **kw):
    from concourse.bass_utils import run_bass_kernel_spmd

    args = {
        k: np.ascontiguousarray(np.asarray(v, dtype=np.float32))
        for k, v in inputs.items()
    }
    nc = _get_nc()
    in_maps = [_prep_core_inputs(c, **args) for c in range(NCORE)]
    res = run_bass_kernel_spmd(
        nc, in_maps, core_ids=list(range(NCORE)), trace=trace, **kw
    )
    out = np.zeros((B, O), np.float32)
    for r in res.results:
        out += r["out"]
    return out, res


def kernel(**inputs):
    out, _ = _run(inputs)
    return out


# revision 16
# speedup vs baseline: 1.0937x; 1.0582x over previous
"""MoE routing kernel for Trainium2, sharded across 8 NeuronCores.

Math (reference): top-2 gated MoE with 6 experts,
  out[b,o] = sum_e gates[b,e] * (relu(x @ w1[e] + b1[e]) @ w2[e] + b2[e])[b,o]

Sharding: the expert computation decomposes into E*H = 384 independent
"hidden units" (unit (e,j) touches only w1[e,:,j] and w2[e,j,:]).  Each of
the 8 cores gets 48 units and produces a gate-weighted partial output
[32,1000]; the partials are summed on the host (the unshard step for a
partial-sum sharding).  Gating (logits -> top2 -> softmax -> dense gates)
is replicated on every core; a per-core 0/1 selection matrix maps gate
columns onto that core's units, so one uniform SPMD program serves all
cores.

Perf notes:
- x^T, gate_w, gate_b, b1 and the selection matrix ride in ONE packed
  [128,361] DMA so the gating chain starts as early as possible; w1 and
  the repacked w2 overlap on the two HW DGE queues (SP + Activation).
- top-2 via the DVE `max` sort unit (one instruction) instead of a
  max/mask/max chain; softmax normalization is deferred into the
  PSUM->SBUF output copies (per-partition 1/den scale), which also
  normalizes the b2 gate rows for free.
- w2 is repacked to [128,500] (column halves stacked at a 64-partition
  offset) so its DMA is dense and both matmul slices are 32-aligned.
- the [32,1000] output is produced in 4 matmul chunks whose copies and
  DMAs pipeline behind the PE, alternating the two DMA queues.
"""

import numpy as np

B, D, H, O, E = 32, 1024, 64, 1000, 6
NCORE = 8
P = 128
KC = D // P          # 8 k-chunks for the D contraction
U = (E * H) // NCORE  # 48 hidden units per core
M = U + 2             # +2 rows carry the b2 bias contributions
E8 = 8                # logits padded to 8 so the DVE sort unit reads PSUM

GOFF = KC * B        # 256: gate_w chunks (8 cols each, last 2 zero)
BOFF = GOFF + KC * E8   # 320: gate_b row (last 2 entries -1e30)
B1OFF = BOFF + E8       # 328: b1 column
SOFF = B1OFF + 1        # 329: sel block
XGW = SOFF + M          # 379 packed input width

_CACHE = {}


def _build_nc():
    import concourse.bacc as bacc
    import concourse.mybir as mybir
    import concourse.tile as tile
    from concourse.masks import make_identity

    f32 = mybir.dt.float32
    Alu = mybir.AluOpType
    Act = mybir.ActivationFunctionType

    nc = bacc.Bacc(
        "TRN2",
        target_bir_lowering=False,
        debug=False,
        enable_asserts=False,
        num_devices=NCORE,
    )

    def din(name, shape):
        return nc.dram_tensor(name, shape, f32, kind="ExternalInput").ap()

    t_xg = din("xg", [P, XGW])        # xt | gws | gbr | b1 | sel (see _prep)
    t_w1s = din("w1s", [P, KC * U])   # per-core w1 cols: [p, k*U+j] = w1[e_j, k*P+p, h_j]
    t_w2p = din("w2p", [P, O // 2])   # rows 0:M = w2s[:, :500]; rows 64:64+M = w2s[:, 500:]
    t_out = nc.dram_tensor("out", [B, O], f32, kind="ExternalOutput").ap()

    with tile.TileContext(nc) as tc:
        with (
            tc.tile_pool(name="sb", bufs=1) as sb,
            tc.tile_pool(name="ps", bufs=1, space="PSUM") as ps,
        ):
            # --- loads: xg first (feeds the whole gating chain), w1s second
            # on the SP queue; w2p alone on the Activation queue ---
            xg = sb.tile([P, XGW], f32)
            nc.sync.dma_start(xg[:], t_xg[:])
            w1s = sb.tile([P, KC * U], f32)
            nc.sync.dma_start(w1s[:], t_w1s[:])
            w2p = sb.tile([P, O // 2], f32)
            nc.sync.dma_start(w2p[:], t_w2p[:])

            gbr = xg[0:1, BOFF:BOFF + E8]
            b1s = xg[0:U, B1OFF:B1OFF + 1]
            sel = xg[0:E, SOFF:SOFF + M]

            # small constants, prepared on idle engines during the DMAs
            ones = sb.tile([1, B], f32)
            nc.gpsimd.memset(ones[:], 1.0)
            R = sb.tile([M, B], f32)     # relu(h+b1) rows; 1.0 in the b2 slots
            nc.gpsimd.memset(R[:], 1.0)
            ident = sb.tile([B, B], f32)
            make_identity(nc, ident[:])

            # logits [B, 8] = x @ gate_w | -1e30 pad (cols 6,7 get 0 from the
            # zero gate_w pad plus -1e30 from the bias row), so the DVE sort
            # unit can read the PSUM tile directly -- no SBUF staging copy
            lg = ps.tile([B, E8], f32)
            for k in range(KC):
                nc.tensor.matmul(
                    lg[:],
                    xg[:, k * B:(k + 1) * B],
                    xg[:, GOFF + k * E8:GOFF + (k + 1) * E8],
                    start=(k == 0),
                    stop=False,
                )
            nc.tensor.matmul(lg[:], ones[:], gbr, start=False, stop=True)

            # hT [U, B] = (w1 cols)^T @ x^T  (PE runs this while the gating
            # chain occupies DVE/ACT)
            ht = ps.tile([U, B], f32)
            for k in range(KC):
                nc.tensor.matmul(
                    ht[:],
                    w1s[:, k * U:(k + 1) * U],
                    xg[:, k * B:(k + 1) * B],
                    start=(k == 0),
                    stop=(k == KC - 1),
                )

            # top-2 gating.  mx[:,0:2] = two largest logits per row (DVE sort
            # unit); wgt = exp(logits) masked to the top-2 (unnormalized --
            # the 1/den softmax scale is applied to the final output instead,
            # which also normalizes the b2 gate rows).
            ex = sb.tile([B, E], f32)
            nc.scalar.activation(ex[:], lg[:, 0:E], Act.Exp, scale=1.0)
            mx = sb.tile([B, 8], f32)
            nc.vector.max(mx[:], lg[:])
            ind = sb.tile([B, E], f32)   # 1.0 at top-2 entries
            nc.vector.tensor_scalar(ind[:], lg[:, 0:E], mx[:, 1:2], None, Alu.is_ge)
            wgt = sb.tile([B, E], f32)
            nc.vector.tensor_mul(wgt[:], ind[:], ex[:])
            den = sb.tile([B, 1], f32)
            nc.vector.tensor_reduce(den[:], wgt[:], axis=mybir.AxisListType.X, op=Alu.add)
            rden = sb.tile([B, 1], f32)
            nc.vector.reciprocal(rden[:], den[:])

            # GT [M, B] = sel^T @ wgt^T : per-unit (unnormalized) gate rows
            gtp = ps.tile([E, B], f32)
            nc.tensor.transpose(gtp[:], wgt[:], ident[:])
            gt = sb.tile([E, B], f32)
            nc.scalar.copy(gt[:], gtp[:])
            GT = ps.tile([M, B], f32)
            nc.tensor.matmul(GT[:], sel, gt[:], start=True, stop=True)

            # R rows :U = relu(hT + b1) (b2 slots stay at their memset 1.0);
            # MT = GT . R is the gated hidden matrix, written at base
            # partitions 0 AND 64 so each output matmul's lhsT shares a base
            # partition with its w2p slice
            nc.gpsimd.tensor_scalar(R[:U, :], ht[:], b1s, 0.0, Alu.add, Alu.max)
            MT = sb.tile([P, B], f32)
            nc.vector.tensor_mul(MT[0:M, :], GT[:], R[:])
            nc.vector.tensor_mul(MT[64:64 + M, :], GT[:], R[:])

            # partial out [B, O] = (MT^T @ w2) * rden.  Two fp32r matmuls of
            # 500 output columns each (fp32r streams 1 row/cycle when the
            # moving free dim >= 256, vs 4 cycles for plain fp32); the two
            # normalizing PSUM->SBUF copies run on different engines (DVE and
            # ACT) and each half goes out on its own DMA queue.
            f32r = mybir.dt.float32r
            osb = sb.tile([B, O], f32)
            NH = O // 2
            for i in range(2):
                r0 = 64 * i
                op = ps.tile([B, NH], f32, tag=f"op{i}", name=f"op{i}")
                nc.tensor.matmul(
                    op[:],
                    MT[r0:r0 + M, :].bitcast(f32r),
                    w2p[r0:r0 + M, :].bitcast(f32r),
                    start=True,
                    stop=True,
                )
                o0 = i * NH
                if i == 0:
                    nc.vector.tensor_scalar_mul(osb[:, o0:o0 + NH], op[:], rden[:])
                    nc.sync.dma_start(t_out[:, o0:o0 + NH], osb[:, o0:o0 + NH])
                else:
                    nc.scalar.mul(osb[:, o0:o0 + NH], op[:], rden[:])
                    nc.scalar.dma_start(t_out[:, o0:o0 + NH], osb[:, o0:o0 + NH])

    nc.compile()
    return nc


def _get_nc():
    if "nc" not in _CACHE:
        _CACHE["nc"] = _build_nc()
    return _CACHE["nc"]


def _prep_core_inputs(c, x, gate_w, gate_b, w1, b1, w2, b2):
    us = np.arange(U * c, U * (c + 1))
    es = us // H
    hs = us % H

    xg = np.zeros((P, XGW), np.float32)
    xg[:, 0:KC * B] = x.reshape(B, KC, P).transpose(2, 1, 0).reshape(P, KC * B)
    gw8 = np.zeros((KC, P, E8), np.float32)
    gw8[:, :, :E] = gate_w.reshape(KC, P, E)
    xg[:, GOFF:GOFF + KC * E8] = gw8.transpose(1, 0, 2).reshape(P, KC * E8)
    xg[0, BOFF:BOFF + E] = gate_b
    xg[0, BOFF + E:BOFF + E8] = -1e30
    xg[0:U, B1OFF] = b1[es, hs]

    sel = np.zeros((E, M), np.float32)
    sel[es, np.arange(U)] = 1.0

    w2s = np.zeros((M, O), np.float32)
    w2s[:U] = w2[es, hs, :]
    owned = [e for e in range(E) if U * c <= e * H < U * (c + 1)]
    for slot, e in enumerate(owned):
        w2s[U + slot] = b2[e]
        sel[e, U + slot] = 1.0
    xg[0:E, SOFF:SOFF + M] = sel

    w2p = np.zeros((P, O // 2), np.float32)
    w2p[0:M] = w2s[:, :O // 2]
    w2p[64:64 + M] = w2s[:, O // 2:]

    w1c = w1[es, :, hs]  # [U, D]
    w1s = np.ascontiguousarray(
        w1c.T.reshape(KC, P, U).transpose(1, 0, 2).reshape(P, KC * U)
    )

    return {"xg": xg, "w1s": w1s, "w2p": w2p}


def _run(inputs, trace=False, **kw):
    from concourse.bass_utils import run_bass_kernel_spmd

    args = {
        k: np.ascontiguousarray(np.asarray(v, dtype=np.float32))
        for k, v in inputs.items()
    }
    nc = _get_nc()
    in_maps = [_prep_core_inputs(c, **args) for c in range(NCORE)]
    res = run_bass_kernel_spmd(
        nc, in_maps, core_ids=list(range(NCORE)), trace=trace, **kw
    )
    out = np.zeros((B, O), np.float32)
    for r in res.results:
        out += r["out"]
    return out, res


def kernel(**inputs):
    out, _ = _run(inputs)
    return out
